# revision 1
# baseline (speedup 1.0000x reference)
"""CoAttention Trainium2 kernel.

Computes A[b,i,j] = u[b,i,:]@w1 + v[b,j,:]@w2 + sum_d u[b,i,d]*w3[d]*v[b,j,d]
for u, v: [16, 2048, 256] f32, w1/w2/w3: [256] f32 -> A: [16, 2048, 2048] f32.

Sharding: batch dim (16) split across 8 NeuronCores (2 batches/core, data
parallel); w1/w2/w3 replicated. Inside each core:
  - load u[b], v[b] in natural layout, PE-transpose to [d, seq] layout
  - uw3 = u * w3 (DVE), transposed -> stationary matmul operand
  - A tile [128 i, 512 j] accumulated in PSUM over 2 d-chunks (fp32r matmuls,
    1 cycle/row at N=512)
  - w2v broadcast tile [128, 2048] built once per batch via PE (w2-bcast lhsT)
  - single fused DVE op per tile: out = (psum + w1u[i]) + w2v_bcast[:, j]
  - 1MB contiguous output DMA per 128-row i-block
"""

import numpy as np
from contextlib import ExitStack

B, S, D = 16, 2048, 256
N_CORES = 8
BPC = B // N_CORES  # batches per core
P = 128
NB = S // P    # 16 seq blocks
NCH = D // P   # 2 contraction chunks
FQ = 512       # matmul free (moving) dim
NQ = S // FQ   # 4 j quarters

_CACHE = {}


def _build(level=40):
    # level: bisect scaffolding. 1=DMA only, 2=+transposes, 3=+matmuls,
    # 35=+setup/w2v matmuls, 40=full
    import concourse.bacc as bacc
    import concourse.mybir as mybir
    import concourse.tile as tile
    from concourse.masks import make_identity

    dt = mybir.dt
    f32 = dt.float32
    f32r = dt.float32r
    ADD = mybir.AluOpType.add
    MULT = mybir.AluOpType.mult

    nc = bacc.Bacc("TRN2", debug=False, num_devices=N_CORES)
    u_d = nc.dram_tensor("u", [BPC, S, D], f32, kind="ExternalInput").ap()
    v_d = nc.dram_tensor("v", [BPC, S, D], f32, kind="ExternalInput").ap()
    w1_d = nc.dram_tensor("w1", [1, D], f32, kind="ExternalInput").ap()
    w2_d = nc.dram_tensor("w2", [1, D], f32, kind="ExternalInput").ap()
    w3_d = nc.dram_tensor("w3", [1, D], f32, kind="ExternalInput").ap()
    out_d = nc.dram_tensor("out", [BPC, S, S], f32, kind="ExternalOutput").ap()

    with tile.TileContext(nc) as tc, ExitStack() as ctx:
        const = ctx.enter_context(tc.tile_pool(name="const", bufs=1))
        inp = ctx.enter_context(tc.tile_pool(name="inp", bufs=2))
        vt_pool = ctx.enter_context(tc.tile_pool(name="vt", bufs=2))
        work = ctx.enter_context(tc.tile_pool(name="work", bufs=2))
        outp = ctx.enter_context(tc.tile_pool(name="outp", bufs=3))
        pst = ctx.enter_context(tc.tile_pool(name="pst", bufs=2, space="PSUM"))
        psa = ctx.enter_context(tc.tile_pool(name="psa", bufs=4, space="PSUM"))

        # ---- constants ----
        if level >= 2:
            ident = const.tile([P, P], f32, tag="ident")
            make_identity(nc, ident[:])
        if level >= 35:
            ones = const.tile([1, P], f32, tag="ones")
            nc.vector.memset(ones[:], 1.0)
            w1r = const.tile([1, D], f32, tag="w1r")
            nc.scalar.dma_start(out=w1r[:], in_=w1_d)
            w2r = const.tile([1, D], f32, tag="w2r")
            nc.scalar.dma_start(out=w2r[:], in_=w2_d)
            w3r = const.tile([1, D], f32, tag="w3r")
            nc.scalar.dma_start(out=w3r[:], in_=w3_d)

            # broadcast w1/w3 across partitions -> [128, 256]
            w1b = const.tile([P, D], f32, tag="w1b")
            w3b = const.tile([P, D], f32, tag="w3b")
            for wrow, wb in ((w1r, w1b), (w3r, w3b)):
                ps = psa.tile([P, D], f32, tag="ps")
                nc.tensor.matmul(
                    ps[:], lhsT=ones[:], rhs=wrow[:], start=True, stop=True
                )
                nc.vector.tensor_copy(wb[:], ps[:])
            # w2T chunks: [d_in_chunk, ch, out_partition] with w2 replicated
            # along the out-partition direction (stationary operand so that
            # psum[p, j] += sum_d w2[d] * vT[d, j] gives the w2v broadcast).
            w2t = const.tile([P, NCH, P], f32r, tag="w2t")
            for ch in range(NCH):
                ps = psa.tile([P, P], f32, tag="ps")
                nc.tensor.matmul(
                    ps[:], lhsT=w2r[:, ch * P:(ch + 1) * P], rhs=ones[:],
                    start=True, stop=True,
                )
                nc.vector.tensor_copy(w2t[:, ch, :], ps[:])

        for bi in range(BPC):
            # whole-batch input loads (2 MB each, single DMA, ACT HWDGE ring
            # so they don't queue behind output stores on the SP ring)
            v_all = inp.tile([P, NB, D], f32, tag="v_all")
            nc.scalar.dma_start(
                out=v_all[:], in_=v_d[bi].rearrange("(nb p) d -> p nb d", p=P)
            )
            u_all = inp.tile([P, NB, D], f32, tag="u_all")
            nc.scalar.dma_start(
                out=u_all[:], in_=u_d[bi].rearrange("(nb p) d -> p nb d", p=P)
            )

            # transpose v -> vT [d_in_chunk, ch, j]
            if level >= 2:
                vt = vt_pool.tile([P, NCH, S], f32r, tag="vt")
                for jb in range(NB):
                    for ch in range(NCH):
                        ps = pst.tile([P, P], f32, tag="pst")
                        nc.tensor.transpose(
                            ps[:], v_all[:, jb, ch * P:(ch + 1) * P], ident[:]
                        )
                        nc.any.tensor_copy(vt[:, ch, jb * P:(jb + 1) * P], ps[:])

            # w2v broadcast [p, j] = sum_d w2[d] * v[j, d]  (same for all p)
            if level >= 35:
                w2vb = work.tile([P, S], f32, tag="w2vb")
                for q in range(NQ):
                    ps = psa.tile([P, FQ], f32, tag="ps")
                    for ch in range(NCH):
                        nc.tensor.matmul(
                            ps[:],
                            lhsT=w2t[:, ch, :],
                            rhs=vt[:, ch, q * FQ:(q + 1) * FQ],
                            start=(ch == 0), stop=(ch == NCH - 1),
                        )
                    nc.any.tensor_copy(w2vb[:, q * FQ:(q + 1) * FQ], ps[:])

            for ib in range(NB):
                u_nat = u_all[:, ib, :]
                if level >= 40:
                    # w1u[i] = sum_d u[i,d] w1[d] (per-partition col);
                    # uw3 = u * w3
                    scr = work.tile([P, D], f32, tag="scr")
                    w1u = work.tile([P, 1], f32, tag="w1u")
                    nc.vector.tensor_tensor(scr[:], u_nat, w1b[:], op=MULT)
                    nc.vector.tensor_reduce(
                        out=w1u[:], in_=scr[:], axis=mybir.AxisListType.X,
                        op=ADD,
                    )
                    uw3 = work.tile([P, D], f32, tag="uw3")
                    nc.vector.tensor_tensor(uw3[:], u_nat, w3b[:], op=MULT)
                    tsrc = uw3
                else:
                    tsrc = u_nat
                # transpose uw3 -> [d_in_chunk, ch, i]
                if level >= 2:
                    uw3t = work.tile([P, NCH, P], f32r, tag="uw3t")
                    for ch in range(NCH):
                        ps = pst.tile([P, P], f32, tag="pst")
                        nc.tensor.transpose(
                            ps[:], tsrc[:, ch * P:(ch + 1) * P], ident[:]
                        )
                        nc.any.tensor_copy(uw3t[:, ch, :], ps[:])

                orow = outp.tile([P, S], f32, tag="orow")
                if level >= 3:
                    pss = [
                        psa.tile([P, FQ], f32, tag="ps", name=f"ps_{bi}_{ib}_{q}")
                        for q in range(NQ)
                    ]
                    # ch-outer: keep the stationary operand loaded across 4 mms
                    for ch in range(NCH):
                        for q in range(NQ):
                            nc.tensor.matmul(
                                pss[q][:],
                                lhsT=uw3t[:, ch, :],
                                rhs=vt[:, ch, q * FQ:(q + 1) * FQ],
                                start=(ch == 0), stop=(ch == NCH - 1),
                            )
                    for q in range(NQ):
                        qs = slice(q * FQ, (q + 1) * FQ)
                        if level >= 40:
                            # orow = (psum + w2v_bcast) on DVE (PSUM read),
                            # then += w1u in place on ACT (SBUF only)
                            nc.vector.tensor_add(
                                out=orow[:, qs], in0=pss[q][:],
                                in1=w2vb[:, qs],
                            )
                            nc.scalar.activation(
                                out=orow[:, qs], in_=orow[:, qs],
                                func=mybir.ActivationFunctionType.Identity,
                                bias=w1u[:], scale=1.0,
                            )
                        else:
                            nc.vector.tensor_copy(orow[:, qs], pss[q][:])
                else:
                    nc.vector.memset(orow[:], float(ib))
                nc.sync.dma_start(
                    out=out_d[bi, ib * P:(ib + 1) * P, :], in_=orow[:]
                )

    nc.compile()
    return nc


def _get_nc():
    if "nc" not in _CACHE:
        _CACHE["nc"] = _build()
    return _CACHE["nc"]


def kernel(u, v, w1, w2, w3, _trace=False, _trace_cores=None, _results_out=None):
    from concourse.bass_utils import run_bass_kernel_spmd

    nc = _get_nc()
    u = np.ascontiguousarray(u, dtype=np.float32)
    v = np.ascontiguousarray(v, dtype=np.float32)
    w1 = np.ascontiguousarray(w1, dtype=np.float32).reshape(1, D)
    w2 = np.ascontiguousarray(w2, dtype=np.float32).reshape(1, D)
    w3 = np.ascontiguousarray(w3, dtype=np.float32).reshape(1, D)

    in_maps = [
        {
            "u": np.ascontiguousarray(u[c * BPC:(c + 1) * BPC]),
            "v": np.ascontiguousarray(v[c * BPC:(c + 1) * BPC]),
            "w1": w1,
            "w2": w2,
            "w3": w3,
        }
        for c in range(N_CORES)
    ]
    kw = {}
    if _trace:
        kw["trace"] = True
        if _trace_cores is not None:
            kw["trace_cores"] = _trace_cores
    res = run_bass_kernel_spmd(nc, in_maps, core_ids=list(range(N_CORES)), **kw)
    if _results_out is not None:
        _results_out.append(res)
    return np.concatenate([res.results[c]["out"] for c in range(N_CORES)], axis=0)

